# revision 1
# baseline (speedup 1.0000x reference)
"""Trainium2 Bass kernel for nn_LocalSelfAttention (point-cloud local attention).

Sharding: 8 cores; core c handles batch b=c//4, query rows (c%4)*1024..+1024.
Per-core pipeline (128-query tiles):
  - d2 to all 4096 points via ACT Square(scale=-1,bias=q_c) on replicated
    coordinate rows + DVE combine (bit-matches reference's (q-p)^2 sum order)
  - exact top-32 via DVE max8/max_index/match_replace rounds (lax.top_k
    semantics incl. stable ties)
  - neighbor gather via gpsimd ap_gather on packed bf16 K/V columns
  - pos-MLP (bf16 PE) fused: h = relu(W1.xyz_nei - W1.xyz_q + b1),
    pos = W2.h + b2
  - logits via DVE prod + PE head-indicator matmul (head-sum with built-in
    replication); softmax w/o max-subtraction (logits are small)
  - value contraction via DVE mult + pool_avg over k; final Wp matmul on PE
"""
import sys
import numpy as np

sys.path.insert(0, "/opt/trn_rl_repo")
sys.path.insert(0, "/opt/trn_rl_repo/concourse")

import concourse.bass as bass
import concourse.tile as tile
from concourse import mybir
from concourse import library_config
from concourse.bass_utils import run_bass_kernel_spmd
from contextlib import ExitStack

B, P, DIM, HEADS, K = 2, 4096, 256, 8, 32
DH = DIM // HEADS
SCALE = float(DH ** -0.5)
NCORES = 8
QPC = P * B // NCORES      # queries per core (1024)
NT = QPC // 128            # query tiles per core (8)
F32 = mybir.dt.float32
BF16 = mybir.dt.bfloat16
U16 = mybir.dt.uint16
I16 = mybir.dt.int16
U32 = mybir.dt.uint32
AF = mybir.ActivationFunctionType
OP = mybir.AluOpType
NEG_INF = -3.0e38


# ---------------------------------------------------------------- tile patch
def _patched_drain_and_barrier(self, tick_clock, wait_clock):
    import bass_rust
    nc = self.nc
    nops = [nc.sync.nop(nofuse=True) for _ in range(24)]
    drain_inst = nc.sync.drain()
    wait_clock.add_sem_waits(
        drain_inst.ins, tile.ScopedClock({None: tick_clock.global_clock})
    )
    si = drain_inst.ins.sync_info
    waits = list(si.on_wait)
    if len(waits) > 1:
        extra = waits[1:]
        assert len(extra) <= len(nops), f"need {len(extra)} wait nops"
        for i, w in enumerate(extra):
            nops[i].ins.sync_info = bass_rust.SyncInfo(on_wait=[w], on_update=[])
        si.on_wait = waits[:1]
    nc.all_engine_barrier()
    assert self.sems is not None
    popped = nc._tile_sem_poison_stack.pop()
    assert popped is self._sem_poison
    nc.clear_and_free_semaphores(list(self.sems.allocated().values()))
    nc.all_engine_barrier()


tile.TileContext._drain_and_barrier = _patched_drain_and_barrier


def split_excess_waits(nc, cap=1):
    """Walrus in this env only encodes a limited number of sem-waits per
    instruction (2 generally, 1 for ldweights-fused matmuls and drains).
    Move excess waits onto single-wait NOPs inserted just before the
    offending instruction (same-engine program order keeps semantics)."""
    import bass_rust
    caps = {"InstDrain": 1, "InstMatmult": 1, "InstMatmultMx": 1, "InstDMACopy": 1}
    all_blocks = [blk for func in nc.m.functions for blk in func.blocks]
    for bb in all_blocks:
        insts = bb.instructions
        i = 0
        while i < len(insts):
            inst = insts[i]
            si = inst.sync_info
            if si is None:
                i += 1
                continue
            waits = list(si.on_wait)
            limit = caps.get(type(inst).__name__, cap)
            if len(waits) <= limit:
                i += 1
                continue
            eng = inst.engine
            keep = waits[:limit]
            extra = waits[limit:]
            nops = []
            for w in extra:
                ni = nc.engines[eng].nop(nofuse=True)
                raw = ni.ins
                for cand in all_blocks:
                    cl = cand.instructions
                    if cl and cl[-1].name == raw.name:
                        cl.pop()
                        break
                raw.sync_info = bass_rust.SyncInfo(on_wait=[w], on_update=[])
                nops.append(raw)
            si.on_wait = keep
            for j, ni in enumerate(nops):
                insts.insert(i + j, ni)
            i += 1 + len(nops)


# ------------------------------------------------------------- program build
_CACHE = {}


def build_program(reps=1):
    key = ("nc", reps)
    if key in _CACHE:
        return _CACHE[key]
    nc = bass.Bass()
    dram = {}
    def din(name, shape, dt=F32):
        dram[name] = nc.dram_tensor(name, shape, dt, kind="ExternalInput")
        return dram[name]

    din("xyz", (P, 3))
    din("feats", (P, DIM))
    din("qxyz", (QPC, 3))
    din("qfeat", (QPC, DIM))
    din("WqT", (DIM, DIM)); din("WkTb", (DIM, DIM), BF16); din("WvTb", (DIM, DIM), BF16)
    din("WpT32", (DIM, DIM))            # Wp.T / 32  (pool_avg folding)
    din("bp_rep", (128, DIM))
    din("W1T", (3, DIM), BF16)
    din("W2T", (DIM, DIM), BF16)
    din("b1c", (DIM, 1)); din("b2c", (DIM, 1))
    din("hind", (4, 128, 128), BF16)    # head-indicator lhsT [t_out*2+dt_in]
    din("ident", (128, 128))            # fp32 identity (transpose)
    din("nident", (128, 128), BF16)     # -identity bf16
    out_d = nc.dram_tensor("out", (QPC, DIM), F32, kind="ExternalOutput")
    dram_scr = nc.dram_tensor("idxscr", (NT, 128 * K), U32, kind="Internal")
    dram_kv = nc.dram_tensor("kvpack", (P, 260), U32, kind="Internal")

    CH = 512            # (q,k) chunk: 16 queries x 32 neighbors
    NCH = P // CH       # 8 chunks per tile

    with tile.TileContext(nc) as tc:
        with ExitStack() as ctx:
            cpool = ctx.enter_context(tc.tile_pool(name="const", bufs=1))
            sb = {}
            for name, shape, dt in [
                ("WpT32", (DIM, DIM), F32),
                ("bp_rep", (128, DIM), F32), ("W1T", (3, DIM), BF16),
                ("W2T", (DIM, DIM), BF16), ("b1c", (DIM, 1), F32),
                ("b2c", (DIM, 1), F32),
                ("ident", (128, 128), F32),
                ("nident", (128, 128), BF16),
            ]:
                t = cpool.tile([min(shape[0], 128), *(
                    [shape[0] // 128 * shape[1]] if shape[0] > 128 else [shape[1]])], dt,
                    tag=name, name="w_" + name)
                if shape[0] > 128:
                    nchunk = shape[0] // 128
                    for i in range(nchunk):
                        nc.sync.dma_start(
                            t[:, i * shape[1]:(i + 1) * shape[1]],
                            dram[name].ap()[i * 128:(i + 1) * 128, :])
                else:
                    nc.sync.dma_start(t[:, :], dram[name].ap())
                sb[name] = t
            hind = cpool.tile([128, 4 * 128], BF16, tag="hind")
            for i in range(4):
                nc.sync.dma_start(hind[:, i * 128:(i + 1) * 128],
                                  dram["hind"].ap()[i])

            def wslice(name, r0, r1, c0, c1):
                t = sb[name]
                ncols = DIM if name not in ("b1c", "b2c") else 1
                chunk = r0 // 128
                return t[r0 - chunk * 128:r1 - chunk * 128,
                         chunk * ncols + c0:chunk * ncols + c1]


            # =============== phase A (transient weights/feats) ===============
            xyzT = cpool.tile([16, P], F32, tag="xyzT")
            xyzrep = [cpool.tile([128, P], F32, tag=f"xyzrep{c}", name=f"xyzrep{c}")
                      for c in range(3)]
            uT = [cpool.tile([128, QPC], BF16, tag=f"uT{i}", name=f"uT{i}")
                  for i in range(2)]
            qTall = [cpool.tile([128, QPC], BF16, tag=f"qTall{i}", name=f"qTall{i}")
                     for i in range(2)]

            with tc.tile_pool(name="phA", bufs=2) as apool, \
                 tc.tile_pool(name="phA_big", bufs=1) as bpool, \
                 tc.tile_pool(name="phA_w", bufs=1) as wpool, \
                 tc.tile_pool(name="phA_ps", bufs=2, space="PSUM") as ppool:
                ident = sb["ident"]
                wtmp = {}
                for name, wdt in (("WqT", F32), ("WkTb", BF16), ("WvTb", BF16)):
                    t = wpool.tile([128, 2 * DIM], wdt, tag=name, name="wa_" + name)
                    for i in range(2):
                        nc.sync.dma_start(t[:, i * DIM:(i + 1) * DIM],
                                          dram[name].ap()[i * 128:(i + 1) * 128, :])
                    wtmp[name] = t

                def wsl(name, r0, r1, c0, c1):
                    t = wtmp[name]
                    chunk = r0 // 128
                    return t[r0 - chunk * 128:r1 - chunk * 128,
                             chunk * DIM + c0:chunk * DIM + c1]

                # featsT [2][128, 4096] fp32 (transient)
                featsT = [bpool.tile([128, P], F32, tag=f"featsT{i}",
                                     name=f"featsT{i}") for i in range(2)]
                for pt in range(P // 128):
                    ft = apool.tile([128, DIM], F32, tag="ft_in")
                    nc.sync.dma_start(ft[:, :],
                                      dram["feats"].ap()[pt * 128:(pt + 1) * 128, :])
                    for et in range(2):
                        ps = ppool.tile([128, 128], F32, tag="tr_ps")
                        nc.tensor.transpose(ps[:, :], ft[:, et * 128:(et + 1) * 128],
                                            ident[:, :])
                        nc.scalar.activation(featsT[et][:, pt * 128:(pt + 1) * 128],
                                             ps[:, :], AF.Identity)
                # qfeatsT [2][128, QPC] (transient)
                qfeatsT = [bpool.tile([128, QPC], F32, tag=f"qfeatsT{i}",
                                      name=f"qfeatsT{i}") for i in range(2)]
                for pt in range(QPC // 128):
                    ft = apool.tile([128, DIM], F32, tag="ft_in")
                    nc.sync.dma_start(ft[:, :],
                                      dram["qfeat"].ap()[pt * 128:(pt + 1) * 128, :])
                    for et in range(2):
                        ps = ppool.tile([128, 128], F32, tag="tr_ps")
                        nc.tensor.transpose(ps[:, :], ft[:, et * 128:(et + 1) * 128],
                                            ident[:, :])
                        nc.scalar.activation(qfeatsT[et][:, pt * 128:(pt + 1) * 128],
                                             ps[:, :], AF.Identity)
                # xyzT rows + replication
                for c in range(3):
                    nc.sync.dma_start(xyzT[c:c + 1, :], dram["xyz"].ap()[:, c:c + 1])
                for c in range(3):
                    nc.sync.dma_start(
                        xyzrep[c][:, :],
                        dram["xyz"].ap()[:, c:c + 1].rearrange(
                            "p one -> (p one)").unsqueeze(0).to_broadcast([128, P]))
                xyzTb = bpool.tile([16, P], BF16, tag="xyzTb")
                nc.scalar.activation(xyzTb[0:3, :], xyzT[0:3, :], AF.Identity)
                qxyzT = bpool.tile([16, QPC], BF16, tag="qxyzT")
                qxyzTf = bpool.tile([16, QPC], F32, tag="qxyzTf")
                for c in range(3):
                    nc.sync.dma_start(qxyzTf[c:c + 1, :],
                                      dram["qxyz"].ap()[:, c:c + 1])
                nc.scalar.activation(qxyzT[0:3, :], qxyzTf[0:3, :], AF.Identity)
                # kv_pack DRAM rows: [4096, 260] u32 = (k_d bf16, v_d bf16) x256 + xyz f32 x3
                # k_full[p, d] = sum_e feats[p, e] Wk[d, e]: lhsT = featsT (bf16), rhs = WkT bf16
                featsTb = [apool.tile([128, P], BF16, tag=f"featsTb{i}",
                                      name=f"featsTb{i}") for i in range(2)]
                for et in range(2):
                    nc.scalar.activation(featsTb[et][:, :], featsT[et][:, :],
                                         AF.Identity)
                for pt in range(P // 128):
                    kvsb = apool.tile([128, 260], U32, tag="kvsb")
                    for wname, half in (("WkTb", 0), ("WvTb", 1)):
                        ps = ppool.tile([128, DIM], F32, tag="kv_ps")
                        for et in range(2):
                            nc.tensor.matmul(
                                ps[:, :],
                                featsTb[et][:, pt * 128:(pt + 1) * 128],
                                wsl(wname, et * 128, (et + 1) * 128, 0, DIM),
                                start=(et == 0), stop=(et == 1))
                        view = kvsb.bitcast(BF16).rearrange(
                            "p (n two) -> p n two", two=2)
                        nc.scalar.activation(view[:, 0:256, half:half + 1],
                                             ps[:, :].unsqueeze(2), AF.Identity)
                    nc.sync.dma_start(
                        kvsb.bitcast(F32)[:, 256:259],
                        dram["xyz"].ap()[pt * 128:(pt + 1) * 128, :])
                    nc.sync.dma_start(dram_kv.ap()[pt * 128:(pt + 1) * 128, :],
                                      kvsb[:, :])
                # uT = W1T @ qxyzT   [2][128, QPC] bf16
                for et in range(2):
                    for chunk in range(QPC // 512):
                        ps = ppool.tile([128, 512], F32, tag="u_ps")
                        nc.tensor.matmul(
                            ps[:, :], sb["W1T"][:, et * 128:(et + 1) * 128],
                            qxyzT[0:3, chunk * 512:(chunk + 1) * 512],
                            start=True, stop=True)
                        nc.scalar.activation(uT[et][:, chunk * 512:(chunk + 1) * 512],
                                             ps[:, :], AF.Identity)
                # qTall = Wq @ qfeats^T  [2][128, QPC] bf16
                for dt_ in range(2):
                    for chunk in range(QPC // 512):
                        ps = ppool.tile([128, 512], F32, tag="q_ps")
                        for et in range(2):
                            nc.tensor.matmul(
                                ps[:, :],
                                wsl("WqT", et * 128, (et + 1) * 128,
                                    dt_ * 128, (dt_ + 1) * 128),
                                qfeatsT[et][:, chunk * 512:(chunk + 1) * 512],
                                start=(et == 0), stop=(et == 1))
                        nc.scalar.activation(
                            qTall[dt_][:, chunk * 512:(chunk + 1) * 512],
                            ps[:, :], AF.Identity)

            # =============== per-tile pipeline ===============
            s_p = ctx.enter_context(tc.tile_pool(name="s", bufs=1))
            sq_p = ctx.enter_context(tc.tile_pool(name="sq", bufs=1))
            tk_p = ctx.enter_context(tc.tile_pool(name="tk", bufs=2))
            g_p = ctx.enter_context(tc.tile_pool(name="gath", bufs=1))
            ck_p = ctx.enter_context(tc.tile_pool(name="chunk", bufs=2))
            sm_p = ctx.enter_context(tc.tile_pool(name="small", bufs=2))
            ps_p = ctx.enter_context(tc.tile_pool(name="ps", bufs=1, space="PSUM"))
            ps_l = ctx.enter_context(tc.tile_pool(name="psl", bufs=2, space="PSUM"))
            ps_t = ctx.enter_context(tc.tile_pool(name="pst", bufs=1, space="PSUM"))

            for t_rep in range(NT * reps):
                t = t_rep % NT
                qs = slice(t * 128, (t + 1) * 128)
                qxyz = sm_p.tile([128, 3], F32, tag="qxyz")
                nc.sync.dma_start(qxyz[:, :], dram["qxyz"].ap()[qs, :])
                # ---- s = -(d2) [128, 4096]
                s = s_p.tile([128, P], F32, tag="s")
                for c in range(3):
                    sq = sq_p.tile([128, P], F32, tag="sq")
                    nc.scalar.activation(sq[:, :], xyzrep[c][:, :], AF.Square,
                                         bias=qxyz[:, c:c + 1], scale=-1.0)
                    if c == 0:
                        nc.vector.tensor_scalar(s[:, :], sq[:, :], -1.0, None,
                                                OP.mult)
                    else:
                        nc.vector.tensor_tensor(s[:, :], s[:, :], sq[:, :],
                                                OP.subtract)
                # ---- top-32
                idx = tk_p.tile([128, K], U16, tag="idx")
                for r in range(4):
                    mx = tk_p.tile([128, 8], F32, tag="mx")
                    nc.vector.max(mx[:, :], s[:, :])
                    nc.vector.max_index(idx[:, r * 8:(r + 1) * 8], mx[:, :], s[:, :])
                    if r < 3:
                        nc.vector.match_replace(s[:, :], mx[:, :], s[:, :], NEG_INF)
                # ---- indices to u32 q-major scratch, read back column-major
                idx32 = tk_p.tile([128, K], U32, tag="idx32")
                nc.vector.tensor_copy(idx32[:, :], idx[:, :])
                nc.sync.dma_start(
                    dram_scr.ap()[t].rearrange("(q k) -> q k", k=K), idx32[:, :])
                idxc = tk_p.tile([128, 32], U32, tag="idxc")
                nc.sync.dma_start(
                    idxc[:, :], dram_scr.ap()[t].rearrange("(m p) -> p m", p=128))
                # ---- gather rows (kv+xyz packed) then transpose to col-major
                g0 = g_p.tile([128, P], U32, tag="g0")   # dims 0-127 (k,v interleaved)
                g1 = g_p.tile([128, P], U32, tag="g1")   # dims 128-255
                kb = [g0.bitcast(BF16).rearrange("p (n two) -> p n two", two=2),
                      g1.bitcast(BF16).rearrange("p (n two) -> p n two", two=2)]
                xpsl = []
                for m in range(32):
                    kvr = g_p.tile([128, 260], U32, tag="kvr")
                    nc.gpsimd.indirect_dma_start(
                        out=kvr[:, :], out_offset=None, in_=dram_kv.ap(),
                        in_offset=bass.IndirectOffsetOnAxis(ap=idxc[:, m:m + 1],
                                                            axis=0))
                    mm = m % 4
                    if mm == 0:
                        pst = [ps_t.tile([128, 512], F32, tag=f"pst{i}",
                                         name=f"pst{i}") for i in range(2)]
                        psx = ps_t.tile([16, 512], F32, tag="psx")
                    kvf = kvr.bitcast(F32)
                    for dt_ in range(2):
                        nc.tensor.transpose(pst[dt_][:, mm * 128:(mm + 1) * 128],
                                            kvf[:, dt_ * 128:(dt_ + 1) * 128],
                                            sb["ident"][:, :])
                    nc.tensor.transpose(psx[0:3, mm * 128:(mm + 1) * 128],
                                        kvf[:, 256:259], sb["ident"][:, :])
                    if mm == 3:
                        ch4 = m // 4
                        c4 = slice(ch4 * 512, (ch4 + 1) * 512)
                        nc.scalar.activation(g0.bitcast(F32)[:, c4], pst[0][:, :],
                                             AF.Identity)
                        nc.scalar.activation(g1.bitcast(F32)[:, c4], pst[1][:, :],
                                             AF.Identity)
                        xpsl.append((ch4, psx))
                xgball = g_p.tile([16, P], BF16, tag="xgball")
                for ch4, psx in xpsl:
                    nc.scalar.activation(
                        xgball[0:3, ch4 * 512:(ch4 + 1) * 512], psx[0:3, :],
                        AF.Identity)
                ov = [sm_p.tile([128, 128], F32, tag=f"ov{i}", name=f"ov{i}")
                      for i in range(2)]
                rz = [sm_p.tile([128, 128], F32, tag=f"rz{i}", name=f"rz{i}")
                      for i in range(2)]
                for ch in range(NCH):
                    cs = slice(ch * CH, (ch + 1) * CH)
                    q16 = slice(t * 128 + ch * 16, t * 128 + (ch + 1) * 16)
                    c16 = slice(ch * 16, (ch + 1) * 16)
                    xgb = xgball
                    # h chunk
                    hc = [ck_p.tile([128, CH], BF16, tag=f"hc{i}", name=f"hc{i}")
                          for i in range(2)]
                    for et in range(2):
                        ps = ps_p.tile([128, CH], F32, tag="h_ps")
                        nc.tensor.matmul(ps[:, :],
                                         sb["W1T"][:, et * 128:(et + 1) * 128],
                                         xgb[0:3, cs], start=True, stop=False)
                        urhs = uT[et][:, q16].unsqueeze(2).to_broadcast(
                            [128, 16, K])
                        nc.tensor.matmul(ps[:, :], sb["nident"][:, :], urhs,
                                         start=False, stop=True)
                        nc.scalar.activation(hc[et][:, :], ps[:, :], AF.Relu,
                                             bias=wslice("b1c", et * 128,
                                                         (et + 1) * 128, 0, 1))
                    # pos chunk [2][128, CH] bf16
                    pos = [ck_p.tile([128, CH], BF16, tag=f"pos{i}", name=f"pos{i}")
                           for i in range(2)]
                    for dt_ in range(2):
                        ps = ps_p.tile([128, CH], F32, tag="pos_ps")
                        for et in range(2):
                            nc.tensor.matmul(
                                ps[:, :],
                                wslice("W2T", et * 128, (et + 1) * 128,
                                       dt_ * 128, (dt_ + 1) * 128),
                                hc[et][:, :], start=(et == 0), stop=(et == 1))
                        nc.scalar.activation(pos[dt_][:, :], ps[:, :], AF.Identity,
                                             bias=wslice("b2c", dt_ * 128,
                                                         (dt_ + 1) * 128, 0, 1))
                    # logits prod (in-place over k-gather view)
                    for dt_ in range(2):
                        kv = kb[dt_][:, cs, 0:1].rearrange("p n one -> p (n one)")
                        nc.vector.tensor_tensor(kv, kv, pos[dt_][:, :], OP.add)
                        kv3 = kv.rearrange("p (a b) -> p a b", b=K)
                        qbc = qTall[dt_][:, q16].unsqueeze(2).to_broadcast(
                            [128, 16, K])
                        nc.vector.tensor_tensor(kv3, kv3, qbc, OP.mult)
                    # head-sum + exp -> attn chunks [2][128, CH] bf16
                    attn = [ck_p.tile([128, CH], BF16, tag=f"attn{i}",
                                      name=f"attn{i}") for i in range(2)]
                    for tout in range(2):
                        ps = ps_l.tile([128, CH], F32, tag="l_ps")
                        for dt_ in range(2):
                            kv = kb[dt_][:, cs, 0:1].rearrange(
                                "p n one -> p (n one)")
                            nc.tensor.matmul(ps[:, :],
                                             hind[:, (tout * 2 + dt_) * 128:
                                                  (tout * 2 + dt_ + 1) * 128],
                                             kv, start=(dt_ == 0), stop=(dt_ == 1))
                        nc.scalar.activation(attn[tout][:, :], ps[:, :], AF.Exp,
                                             scale=SCALE)
                    # Z and values
                    for dt_ in range(2):
                        nc.vector.reduce_sum(rz[dt_][:, c16], attn[dt_][:, :].rearrange(
                            "p (a b) -> p a b", b=K), axis=mybir.AxisListType.X)
                        vv = kb[dt_][:, cs, 1:2].rearrange("p n one -> p (n one)")
                        nc.vector.tensor_tensor(vv, vv, pos[dt_][:, :], OP.add)
                        veffc = ck_p.tile([128, CH], BF16, tag="veffc")
                        nc.vector.tensor_tensor(veffc[:, :], vv, attn[dt_][:, :],
                                                OP.mult)
                        nc.vector.reduce_sum(ov[dt_][:, c16], veffc[:, :].rearrange(
                            "p (a b) -> p a b", b=K), axis=mybir.AxisListType.X)
                # normalize + final projection
                pso = ps_p.tile([128, DIM], F32, tag="o_ps")
                for dt_ in range(2):
                    nc.vector.reciprocal(rz[dt_][:, :], rz[dt_][:, :])
                    nc.vector.tensor_tensor(ov[dt_][:, :], ov[dt_][:, :],
                                            rz[dt_][:, :], OP.mult)
                    nc.tensor.matmul(pso[:, :], ov[dt_][:, :],
                                     wslice("WpT32", dt_ * 128, (dt_ + 1) * 128,
                                            0, DIM),
                                     start=(dt_ == 0), stop=(dt_ == 1))
                osb = sm_p.tile([128, DIM], F32, tag="osb")
                nc.vector.tensor_tensor(osb[:, :], pso[:, :], sb["bp_rep"][:, :],
                                        OP.add)
                nc.sync.dma_start(out_d.ap()[qs, :], osb[:, :])
    split_excess_waits(nc)
    _CACHE[key] = nc
    return nc


def _host_inputs(inputs, core):
    b, qpart = core // 4, core % 4
    qoff = qpart * QPC
    xyz = np.ascontiguousarray(inputs["xyz"][b], np.float32)
    feats = np.ascontiguousarray(inputs["feats"][b], np.float32)
    hind = np.zeros((4, 128, 128), np.float32)
    d_idx = np.arange(128)
    c_idx = np.arange(128)
    for tout in range(2):
        for dtin in range(2):
            gh = (dtin * 128 + d_idx) // DH
            hc = c_idx // DH + 4 * tout
            hind[tout * 2 + dtin] = (gh[:, None] == hc[None, :]).astype(np.float32)
    import ml_dtypes
    bf = lambda x: np.asarray(x, dtype=ml_dtypes.bfloat16)
    return {
        "xyz": xyz, "feats": feats,
        "qxyz": np.ascontiguousarray(xyz[qoff:qoff + QPC], np.float32),
        "qfeat": np.ascontiguousarray(feats[qoff:qoff + QPC], np.float32),
        "WqT": np.ascontiguousarray(inputs["Wq"].T, np.float32),
        "WkTb": bf(inputs["Wk"].T),
        "WvTb": bf(inputs["Wv"].T),
        "WpT32": np.ascontiguousarray(inputs["Wp"].T, np.float32),
        "bp_rep": np.tile(inputs["bp"][None, :], (128, 1)).astype(np.float32),
        "W1T": bf(inputs["W1"].T),
        "W2T": bf(inputs["W2"].T),
        "b1c": np.ascontiguousarray(inputs["b1"][:, None], np.float32),
        "b2c": np.ascontiguousarray(inputs["b2"][:, None], np.float32),
        "hind": bf(hind),
        "ident": np.eye(128, dtype=np.float32),
        "nident": bf(-np.eye(128)),
    }


def kernel(**inputs):
    nc = build_program()
    in_maps = [_host_inputs(inputs, c) for c in range(NCORES)]
    res = run_bass_kernel_spmd(nc, in_maps, list(range(NCORES)))
    out = np.zeros((B, P, DIM), np.float32)
    for c in range(NCORES):
        b, qpart = c // 4, c % 4
        out[b, qpart * QPC:(qpart + 1) * QPC] = res.results[c]["out"]
    return out



# revision 12
# speedup vs baseline: 2.9950x; 2.9950x over previous
"""Trainium2 Bass kernel for nn_LocalSelfAttention (point-cloud local attention).

Sharding: 8 cores; core c handles batch b=c//4, query rows (c%4)*1024..+1024.
Per-core pipeline (128-query tiles):
  - s = 2*q.c - |c|^2 (centered coords) via exact-fp32 PE matmul with a 4-row
    contraction [2cx,2cy,2cz,1] x [cx,cy,cz,-|c|^2]; ranks identically to -d2
    up to fp32 rounding (validated: 1/4096 queries flip a boundary neighbor)
  - top-32 hierarchical: per-128-segment top-8 (DVE max8) -> 256 candidates,
    exact top-32 of candidates, index resolution via a u16 idxtab DRAM
    roundtrip + per-partition indirect-DMA gather
  - neighbor gather of K/V tables + replicated-xyz via gpsimd indirect_copy
    (SBUF-native free-axis gather, no DMA roundtrip, no transposes)
  - pos-MLP (bf16 PE) fused: h = relu(W1.xyz_nei - W1.xyz_q + b1),
    pos = W2.h + b2
  - logits via DVE add/mult + PE head-indicator matmul; softmax w/o
    max-subtraction (logits are small)
  - value contraction via DVE in-place add/mult + reduce; final Wp matmul on PE
"""
import sys
import numpy as np

sys.path.insert(0, "/opt/trn_rl_repo")
sys.path.insert(0, "/opt/trn_rl_repo/concourse")

import concourse.bass as bass
import concourse.tile as tile
from concourse import mybir
from concourse.bass_utils import run_bass_kernel_spmd
from contextlib import ExitStack

B, P, DIM, HEADS, K = 2, 4096, 256, 8, 32
DH = DIM // HEADS
SCALE = float(DH ** -0.5)
NCORES = 8
QPC = P * B // NCORES      # queries per core (1024)
NT = QPC // 128            # query tiles per core (8)
NSEG = 32                  # top-k segments (128 points each)
NCAND = NSEG * 8           # 256 candidates
F32 = mybir.dt.float32
BF16 = mybir.dt.bfloat16
U16 = mybir.dt.uint16
I16 = mybir.dt.int16
U32 = mybir.dt.uint32
AF = mybir.ActivationFunctionType
OP = mybir.AluOpType
NEG_INF = -3.0e38
CH = 512                   # (q,k) chunk: 16 queries x 32 neighbors
NCH = P // CH              # 8 chunks per tile


# ---------------------------------------------------------------- tile patch
def _patched_drain_and_barrier(self, tick_clock, wait_clock):
    import bass_rust
    nc = self.nc
    nops = [nc.sync.nop(nofuse=True) for _ in range(24)]
    drain_inst = nc.sync.drain()
    wait_clock.add_sem_waits(
        drain_inst.ins, tile.ScopedClock({None: tick_clock.global_clock})
    )
    si = drain_inst.ins.sync_info
    waits = list(si.on_wait)
    if len(waits) > 1:
        extra = waits[1:]
        assert len(extra) <= len(nops), f"need {len(extra)} wait nops"
        for i, w in enumerate(extra):
            nops[i].ins.sync_info = bass_rust.SyncInfo(on_wait=[w], on_update=[])
        si.on_wait = waits[:1]
    nc.all_engine_barrier()
    assert self.sems is not None
    popped = nc._tile_sem_poison_stack.pop()
    assert popped is self._sem_poison
    nc.clear_and_free_semaphores(list(self.sems.allocated().values()))
    nc.all_engine_barrier()


tile.TileContext._drain_and_barrier = _patched_drain_and_barrier


def split_excess_waits(nc, cap=1):
    """Walrus in this env only encodes a limited number of sem-waits per
    instruction (2 generally, 1 for ldweights-fused matmuls and drains).
    Move excess waits onto single-wait NOPs inserted just before the
    offending instruction (same-engine program order keeps semantics)."""
    import bass_rust
    caps = {"InstDrain": 1, "InstMatmult": 1, "InstMatmultMx": 1, "InstDMACopy": 1}
    all_blocks = [blk for func in nc.m.functions for blk in func.blocks]
    for bb in all_blocks:
        insts = bb.instructions
        i = 0
        while i < len(insts):
            inst = insts[i]
            si = inst.sync_info
            if si is None:
                i += 1
                continue
            waits = list(si.on_wait)
            limit = caps.get(type(inst).__name__, cap)
            if len(waits) <= limit:
                i += 1
                continue
            eng = inst.engine
            keep = waits[:limit]
            extra = waits[limit:]
            nops = []
            for w in extra:
                ni = nc.engines[eng].nop(nofuse=True)
                raw = ni.ins
                for cand in all_blocks:
                    cl = cand.instructions
                    if cl and cl[-1].name == raw.name:
                        cl.pop()
                        break
                raw.sync_info = bass_rust.SyncInfo(on_wait=[w], on_update=[])
                nops.append(raw)
            si.on_wait = keep
            for j, ni in enumerate(nops):
                insts.insert(i + j, ni)
            i += 1 + len(nops)


# ------------------------------------------------------------- program build
_CACHE = {}


def build_program(reps=1):
    key = ("nc", reps)
    if key in _CACHE:
        return _CACHE[key]
    nc = bass.Bass()
    dram = {}

    def din(name, shape, dt=F32):
        dram[name] = nc.dram_tensor(name, shape, dt, kind="ExternalInput")
        return dram[name]

    din("featsb", (P, DIM), BF16)
    din("qfeatsb", (QPC, DIM), BF16)
    din("rhs3", (3, P))                 # centered xyz^T rows (fp32)
    din("qlhsT4", (4, QPC))             # [2cx,2cy,2cz,1] of this core's queries
    din("xyzTb128", (128, P), BF16)     # centered xyz^T bf16, tiled 8x16 rows
    din("qxyzTb", (16, QPC), BF16)      # centered query xyz^T bf16 (rows 0-2)
    din("WqTb", (DIM, DIM), BF16)
    din("WkTb", (DIM, DIM), BF16)
    din("WvTb", (DIM, DIM), BF16)
    din("WpT", (DIM, DIM))
    din("bp_rep", (128, DIM))
    din("W1T", (3, DIM), BF16)
    din("W2T", (DIM, DIM), BF16)
    din("b1c", (DIM, 1))
    din("b2c", (DIM, 1))
    din("hind", (4, 128, 128), BF16)    # head-indicator lhsT [tout*2+dt_in]
    din("nident", (128, 128), BF16)     # -identity bf16
    din("offtab", (128, NCAND), U16)    # seg(m)*128 offsets
    din("qoff256", (128, K), U16)       # q*256 per partition, tiled K cols
    out_d = nc.dram_tensor("out", (QPC, DIM), F32, kind="ExternalOutput")
    idxtab_d = nc.dram_tensor("idxtab", (NT * 128 * NCAND, 1), U16,
                              kind="Internal")
    scr16 = nc.dram_tensor("idxscr", (NT, 128 * K), U16, kind="Internal")

    with tile.TileContext(nc) as tc:
        with ExitStack() as ctx:
            cpool = ctx.enter_context(tc.tile_pool(name="const", bufs=1))
            sb = {}
            for name, shape, dt in [
                ("WpT", (DIM, DIM), F32),
                ("bp_rep", (128, DIM), F32),
                ("W2T", (DIM, DIM), BF16), ("b1c", (DIM, 1), F32),
                ("b2c", (DIM, 1), F32),
                ("nident", (128, 128), BF16),
                ("xyzTb128", (128, P), BF16),
                ("offtab", (128, NCAND), U16),
                ("qoff256", (128, K), U16),
            ]:
                t = cpool.tile([min(shape[0], 128), *(
                    [shape[0] // 128 * shape[1]] if shape[0] > 128 else [shape[1]])],
                    dt, tag=name, name="w_" + name)
                if shape[0] > 128:
                    nchunk = shape[0] // 128
                    for i in range(nchunk):
                        nc.sync.dma_start(
                            t[:, i * shape[1]:(i + 1) * shape[1]],
                            dram[name].ap()[i * 128:(i + 1) * 128, :])
                else:
                    nc.sync.dma_start(t[:, :], dram[name].ap())
                sb[name] = t
            hind = cpool.tile([128, 4 * 128], BF16, tag="hind")
            for i in range(4):
                nc.sync.dma_start(hind[:, i * 128:(i + 1) * 128],
                                  dram["hind"].ap()[i])
            W1T = cpool.tile([3, DIM], BF16, tag="W1T")
            nc.sync.dma_start(W1T[:, :], dram["W1T"].ap())
            qlhsT4 = cpool.tile([4, QPC], F32, tag="qlhsT4")
            nc.sync.dma_start(qlhsT4[:, :], dram["qlhsT4"].ap())
            rhs4 = cpool.tile([4, P], F32, tag="rhs4")
            nc.sync.dma_start(rhs4[0:3, :], dram["rhs3"].ap())
            ones3 = cpool.tile([3, 1], F32, tag="ones3")
            nc.vector.memset(ones3[:, :], 1.0)
            Ktab = [cpool.tile([128, P], BF16, tag=f"Ktab{h}", name=f"Ktab{h}")
                    for h in range(2)]
            Vtab = [cpool.tile([128, P], BF16, tag=f"Vtab{h}", name=f"Vtab{h}")
                    for h in range(2)]
            uT = [cpool.tile([128, QPC], BF16, tag=f"uT{i}", name=f"uT{i}")
                  for i in range(2)]
            qTall = [cpool.tile([128, QPC], BF16, tag=f"qTall{i}",
                                name=f"qTall{i}") for i in range(2)]

            def wslice(name, r0, r1, c0, c1):
                t = sb[name]
                ncols = DIM if name not in ("b1c", "b2c") else 1
                chunk = r0 // 128
                return t[r0 - chunk * 128:r1 - chunk * 128,
                         chunk * ncols + c0:chunk * ncols + c1]

            # =============== phase A (transient) ===============
            with tc.tile_pool(name="phA_w", bufs=1) as wpool, \
                 tc.tile_pool(name="phA_big", bufs=1) as bpool, \
                 tc.tile_pool(name="phA", bufs=2) as apool, \
                 tc.tile_pool(name="phA_ps", bufs=2, space="PSUM") as ppool:
                wtmp = {}
                for wname in ("WqTb", "WkTb", "WvTb"):
                    t = wpool.tile([128, 2 * DIM], BF16, tag=wname,
                                   name="wa_" + wname)
                    for i in range(2):
                        nc.sync.dma_start(t[:, i * DIM:(i + 1) * DIM],
                                          dram[wname].ap()[i * 128:(i + 1) * 128, :])
                    wtmp[wname] = t

                def wsl(name, r0, r1, c0, c1):
                    t = wtmp[name]
                    chunk = r0 // 128
                    return t[r0 - chunk * 128:r1 - chunk * 128,
                             chunk * DIM + c0:chunk * DIM + c1]

                featsTb = [bpool.tile([128, P], BF16, tag=f"featsTb{i}",
                                      name=f"featsTb{i}") for i in range(2)]
                qfeatsTb = [bpool.tile([128, QPC], BF16, tag=f"qfeatsTb{i}",
                                       name=f"qfeatsTb{i}") for i in range(2)]
                for et in range(2):
                    nc.sync.dma_start_transpose(
                        featsTb[et][:, :],
                        dram["featsb"].ap()[:, et * 128:(et + 1) * 128])
                    nc.sync.dma_start_transpose(
                        qfeatsTb[et][:, :],
                        dram["qfeatsb"].ap()[:, et * 128:(et + 1) * 128])
                qxyzTb = bpool.tile([16, QPC], BF16, tag="qxyzTb")
                nc.sync.dma_start(qxyzTb[:, :], dram["qxyzTb"].ap())

                # rhs4 row 3 = -|c|^2 via ACT square + ones-matmul
                sq3 = bpool.tile([3, P], F32, tag="sq3")
                nc.scalar.activation(sq3[:, :], rhs4[0:3, :], AF.Square)
                for ch in range(8):
                    ps = ppool.tile([1, 512], F32, tag="n2_ps")
                    nc.tensor.matmul(ps[:, :], ones3[:, :],
                                     sq3[:, ch * 512:(ch + 1) * 512],
                                     start=True, stop=True)
                    nc.scalar.activation(rhs4[3:4, ch * 512:(ch + 1) * 512],
                                         ps[:, :], AF.Identity, scale=-1.0)

                # K/V tables [d, p] bf16
                for tabs, wname in ((Ktab, "WkTb"), (Vtab, "WvTb")):
                    for h in range(2):
                        for ch in range(8):
                            ps = ppool.tile([128, 512], F32, tag="kv_ps")
                            for et in range(2):
                                nc.tensor.matmul(
                                    ps[:, :],
                                    wsl(wname, et * 128, (et + 1) * 128,
                                        h * 128, (h + 1) * 128),
                                    featsTb[et][:, ch * 512:(ch + 1) * 512],
                                    start=(et == 0), stop=(et == 1))
                            nc.scalar.activation(
                                tabs[h][:, ch * 512:(ch + 1) * 512], ps[:, :],
                                AF.Identity)
                # uT = W1 @ q_xyz^T
                for h in range(2):
                    for ch in range(QPC // 512):
                        ps = ppool.tile([128, 512], F32, tag="u_ps")
                        nc.tensor.matmul(ps[:, :],
                                         W1T[:, h * 128:(h + 1) * 128],
                                         qxyzTb[0:3, ch * 512:(ch + 1) * 512],
                                         start=True, stop=True)
                        nc.scalar.activation(uT[h][:, ch * 512:(ch + 1) * 512],
                                             ps[:, :], AF.Identity)
                # qTall = Wq @ qfeats^T
                for dt_ in range(2):
                    for ch in range(QPC // 512):
                        ps = ppool.tile([128, 512], F32, tag="q_ps")
                        for et in range(2):
                            nc.tensor.matmul(
                                ps[:, :],
                                wsl("WqTb", et * 128, (et + 1) * 128,
                                    dt_ * 128, (dt_ + 1) * 128),
                                qfeatsTb[et][:, ch * 512:(ch + 1) * 512],
                                start=(et == 0), stop=(et == 1))
                        nc.scalar.activation(
                            qTall[dt_][:, ch * 512:(ch + 1) * 512],
                            ps[:, :], AF.Identity)

            # =============== per-tile pipeline ===============
            s_p = ctx.enter_context(tc.tile_pool(name="s", bufs=2))
            tk_p = ctx.enter_context(tc.tile_pool(name="tk", bufs=2))
            g_p = ctx.enter_context(tc.tile_pool(name="gath", bufs=2))
            xg_p = ctx.enter_context(tc.tile_pool(name="xgath", bufs=1))
            ck_p = ctx.enter_context(tc.tile_pool(name="chunk", bufs=2))
            sm_p = ctx.enter_context(tc.tile_pool(name="small", bufs=2))
            ps_s = ctx.enter_context(tc.tile_pool(name="pss", bufs=2,
                                                  space="PSUM"))
            ps_p = ctx.enter_context(tc.tile_pool(name="psp", bufs=2,
                                                  space="PSUM"))
            ps_l = ctx.enter_context(tc.tile_pool(name="psl", bufs=2,
                                                  space="PSUM"))
            ps_o = ctx.enter_context(tc.tile_pool(name="pso", bufs=1,
                                                  space="PSUM"))

            for t_rep in range(NT * reps):
                t = t_rep % NT
                qsl = slice(t * 128, (t + 1) * 128)
                # ---- s = 2 q.c - |c|^2  [128, 4096] fp32 via PE
                s = s_p.tile([128, P], F32, tag="s")
                for ch in range(8):
                    ps = ps_s.tile([128, 512], F32, tag="s_ps")
                    nc.tensor.matmul(ps[:, :], qlhsT4[:, qsl],
                                     rhs4[:, ch * 512:(ch + 1) * 512],
                                     start=True, stop=True)
                    nc.scalar.activation(s[:, ch * 512:(ch + 1) * 512],
                                         ps[:, :], AF.Identity)
                # ---- hierarchical top-32
                cands = tk_p.tile([128, NCAND], F32, tag="cands")
                idxtab = tk_p.tile([128, NCAND], U16, tag="idxtab")
                for g in range(NSEG):
                    nc.vector.max(cands[:, g * 8:(g + 1) * 8],
                                  s[:, g * 128:(g + 1) * 128])
                for g in range(NSEG):
                    nc.vector.max_index(idxtab[:, g * 8:(g + 1) * 8],
                                        cands[:, g * 8:(g + 1) * 8],
                                        s[:, g * 128:(g + 1) * 128])
                nc.vector.tensor_tensor(idxtab[:, :], idxtab[:, :],
                                        sb["offtab"][:, :], OP.add)
                nc.sync.dma_start(
                    idxtab_d.ap()[t * 128 * NCAND:(t + 1) * 128 * NCAND, 0]
                    .rearrange("(q c) -> q c", c=NCAND), idxtab[:, :])
                slots = tk_p.tile([128, K], U16, tag="slots")
                for r in range(4):
                    mx = tk_p.tile([128, 8], F32, tag="mx")
                    nc.vector.max(mx[:, :], cands[:, :])
                    nc.vector.max_index(slots[:, r * 8:(r + 1) * 8], mx[:, :],
                                        cands[:, :])
                    if r < 3:
                        nc.vector.match_replace(cands[:, :], mx[:, :],
                                                cands[:, :], NEG_INF)
                slotsq = tk_p.tile([128, K], U16, tag="slotsq")
                nc.vector.tensor_tensor(slotsq[:, :], slots[:, :],
                                        sb["qoff256"][:, :], OP.add)
                gidx = tk_p.tile([128, K], U16, tag="gidx")
                nc.gpsimd.indirect_dma_start(
                    out=gidx[:, :], out_offset=None, in_=idxtab_d.ap(),
                    in_offset=bass.IndirectOffsetOnAxis(ap=slotsq[:, :], axis=0),
                    element_offset=t * 128 * NCAND)
                nc.sync.dma_start(
                    scr16.ap()[t].rearrange("(q k) -> q k", k=K), gidx[:, :])
                wr16 = tk_p.tile([16, 128 * K // 16], U16, tag="wr16")
                nc.sync.dma_start(
                    wr16[:, :], scr16.ap()[t].rearrange("(m r) -> r m", r=16))
                wrap = tk_p.tile([128, 128 * K // 16], U16, tag="wrap")
                for blk in range(8):
                    nc.sync.dma_start(wrap[blk * 16:(blk + 1) * 16, :],
                                      wr16[:, :])
                # ---- gathers (SBUF-native, gpsimd)
                kg = [g_p.tile([128, P], BF16, tag=f"kg{h}", name=f"kg{h}")
                      for h in range(2)]
                vg = [g_p.tile([128, P], BF16, tag=f"vg{h}", name=f"vg{h}")
                      for h in range(2)]
                xg = xg_p.tile([128, P], BF16, tag="xg")
                for h in range(2):
                    nc.gpsimd.indirect_copy(kg[h][:, :], Ktab[h][:, :],
                                            wrap[:, :], True)
                    nc.gpsimd.indirect_copy(vg[h][:, :], Vtab[h][:, :],
                                            wrap[:, :], True)
                nc.gpsimd.indirect_copy(xg[:, :], sb["xyzTb128"][:, :],
                                        wrap[:, :], True)
                # ---- chunks
                ov = [sm_p.tile([128, 128], F32, tag=f"ov{i}", name=f"ov{i}")
                      for i in range(2)]
                rz = [sm_p.tile([128, 128], F32, tag=f"rz{i}", name=f"rz{i}")
                      for i in range(2)]
                for ch in range(NCH):
                    cs = slice(ch * CH, (ch + 1) * CH)
                    q16 = slice(t * 128 + ch * 16, t * 128 + (ch + 1) * 16)
                    c16 = slice(ch * 16, (ch + 1) * 16)
                    # h chunk
                    hc = [ck_p.tile([128, CH], BF16, tag=f"hc{i}", name=f"hc{i}")
                          for i in range(2)]
                    for et in range(2):
                        ps = ps_p.tile([128, CH], F32, tag="hp_ps")
                        nc.tensor.matmul(ps[:, :],
                                         W1T[:, et * 128:(et + 1) * 128],
                                         xg[0:3, cs], start=True, stop=False)
                        urhs = uT[et][:, q16].unsqueeze(2).to_broadcast(
                            [128, 16, K])
                        nc.tensor.matmul(ps[:, :], sb["nident"][:, :], urhs,
                                         start=False, stop=True)
                        nc.scalar.activation(hc[et][:, :], ps[:, :], AF.Relu,
                                             bias=wslice("b1c", et * 128,
                                                         (et + 1) * 128, 0, 1))
                    # pos chunk [2][128, CH] bf16
                    pos = [ck_p.tile([128, CH], BF16, tag=f"pos{i}",
                                     name=f"pos{i}") for i in range(2)]
                    for dt_ in range(2):
                        ps = ps_p.tile([128, CH], F32, tag="hp_ps",
                                       name="pos_ps")
                        for et in range(2):
                            nc.tensor.matmul(
                                ps[:, :],
                                wslice("W2T", et * 128, (et + 1) * 128,
                                       dt_ * 128, (dt_ + 1) * 128),
                                hc[et][:, :], start=(et == 0), stop=(et == 1))
                        nc.scalar.activation(pos[dt_][:, :], ps[:, :],
                                             AF.Identity,
                                             bias=wslice("b2c", dt_ * 128,
                                                         (dt_ + 1) * 128, 0, 1))
                    # logits: keff = k + pos, prod = keff * q (in-place on kg)
                    for dt_ in range(2):
                        kk = kg[dt_][:, cs]
                        nc.vector.tensor_tensor(kk, kk, pos[dt_][:, :], OP.add)
                        kk3 = kk.rearrange("p (a b) -> p a b", b=K)
                        qbc = qTall[dt_][:, q16].unsqueeze(2).to_broadcast(
                            [128, 16, K])
                        nc.vector.tensor_tensor(kk3, kk3, qbc, OP.mult)
                    # head-sum + exp -> attn chunks [2][128, CH] bf16
                    attn = [ck_p.tile([128, CH], BF16, tag=f"attn{i}",
                                      name=f"attn{i}") for i in range(2)]
                    for tout in range(2):
                        ps = ps_l.tile([128, CH], F32, tag="l_ps")
                        for dt_ in range(2):
                            nc.tensor.matmul(
                                ps[:, :],
                                hind[:, (tout * 2 + dt_) * 128:
                                     (tout * 2 + dt_ + 1) * 128],
                                kg[dt_][:, cs], start=(dt_ == 0),
                                stop=(dt_ == 1))
                        nc.scalar.activation(attn[tout][:, :], ps[:, :], AF.Exp,
                                             scale=SCALE)
                    # Z and values (in-place on vg)
                    for dt_ in range(2):
                        nc.vector.reduce_sum(
                            rz[dt_][:, c16],
                            attn[dt_][:, :].rearrange("p (a b) -> p a b", b=K),
                            axis=mybir.AxisListType.X)
                        vv = vg[dt_][:, cs]
                        nc.vector.tensor_tensor(vv, vv, pos[dt_][:, :], OP.add)
                        nc.vector.tensor_tensor(vv, vv, attn[dt_][:, :],
                                                OP.mult)
                        nc.vector.reduce_sum(
                            ov[dt_][:, c16],
                            vv.rearrange("p (a b) -> p a b", b=K),
                            axis=mybir.AxisListType.X)
                # normalize + final projection
                pso = ps_o.tile([128, DIM], F32, tag="o_ps")
                for dt_ in range(2):
                    nc.vector.reciprocal(rz[dt_][:, :], rz[dt_][:, :])
                    nc.vector.tensor_tensor(ov[dt_][:, :], ov[dt_][:, :],
                                            rz[dt_][:, :], OP.mult)
                    nc.tensor.matmul(pso[:, :], ov[dt_][:, :],
                                     wslice("WpT", dt_ * 128, (dt_ + 1) * 128,
                                            0, DIM),
                                     start=(dt_ == 0), stop=(dt_ == 1))
                osb = sm_p.tile([128, DIM], F32, tag="osb")
                nc.vector.tensor_tensor(osb[:, :], pso[:, :],
                                        sb["bp_rep"][:, :], OP.add)
                nc.sync.dma_start(out_d.ap()[qsl, :], osb[:, :])
    split_excess_waits(nc)
    _CACHE[key] = nc
    return nc


def _host_inputs(inputs, core):
    import ml_dtypes
    bf = lambda x: np.ascontiguousarray(np.asarray(x, dtype=ml_dtypes.bfloat16))
    b, qpart = core // 4, core % 4
    qoff = qpart * QPC
    xyz = np.asarray(inputs["xyz"][b], np.float32)
    cx = (xyz - np.float32(0.5)).astype(np.float32)       # centered coords
    feats = np.asarray(inputs["feats"][b], np.float32)
    cxq = cx[qoff:qoff + QPC]
    qlhsT4 = np.empty((4, QPC), np.float32)
    qlhsT4[0:3] = 2.0 * cxq.T
    qlhsT4[3] = 1.0
    xyzTb16 = np.zeros((16, P), np.float32)
    xyzTb16[0:3] = cx.T
    xyzTb128 = np.tile(xyzTb16, (8, 1))
    qxyzTb = np.zeros((16, QPC), np.float32)
    qxyzTb[0:3] = cxq.T
    hind = np.zeros((4, 128, 128), np.float32)
    d_idx = np.arange(128)
    c_idx = np.arange(128)
    for tout in range(2):
        for dtin in range(2):
            gh = (dtin * 128 + d_idx) // DH
            hc = c_idx // DH + 4 * tout
            hind[tout * 2 + dtin] = (gh[:, None] == hc[None, :]).astype(
                np.float32)
    offtab = np.tile(
        (np.arange(NCAND, dtype=np.uint16) // 8 * 128)[None, :], (128, 1))
    qoff256 = np.tile((np.arange(128, dtype=np.uint16) * NCAND)[:, None],
                      (1, K))
    return {
        "featsb": bf(feats),
        "qfeatsb": bf(feats[qoff:qoff + QPC]),
        "rhs3": np.ascontiguousarray(cx.T),
        "qlhsT4": np.ascontiguousarray(qlhsT4),
        "xyzTb128": bf(xyzTb128),
        "qxyzTb": bf(qxyzTb),
        "WqTb": bf(inputs["Wq"].T),
        "WkTb": bf(inputs["Wk"].T),
        "WvTb": bf(inputs["Wv"].T),
        "WpT": np.ascontiguousarray(inputs["Wp"].T, np.float32),
        "bp_rep": np.tile(inputs["bp"][None, :], (128, 1)).astype(np.float32),
        "W1T": bf(inputs["W1"].T),
        "W2T": bf(inputs["W2"].T),
        "b1c": np.ascontiguousarray(inputs["b1"][:, None], np.float32),
        "b2c": np.ascontiguousarray(inputs["b2"][:, None], np.float32),
        "hind": bf(hind),
        "nident": bf(-np.eye(128)),
        "offtab": np.ascontiguousarray(offtab),
        "qoff256": np.ascontiguousarray(qoff256),
    }


def kernel(**inputs):
    nc = build_program()
    in_maps = [_host_inputs(inputs, c) for c in range(NCORES)]
    res = run_bass_kernel_spmd(nc, in_maps, list(range(NCORES)))
    out = np.zeros((B, P, DIM), np.float32)
    for c in range(NCORES):
        b, qpart = c // 4, c % 4
        out[b, qpart * QPC:(qpart + 1) * QPC] = res.results[c]["out"]
    return out
